# revision 16
# baseline (speedup 1.0000x reference)
"""MoE expert-FFN (nn_Experts) Trainium2 kernel.

Expert-parallel: one expert per NeuronCore (E = 8 = n_cores).
Host does the token gather (dispatch) and weighted scatter-add (combine);
each core runs the fused FFN for its expert:

    hT = gelu_tanh(W1^T @ tokT + b1)        # [F, C] on chip, f32r matmuls
    out = (hT^T @ W2) * w[:, None]          # [C, D], combine weight fused
                                            # into the PSUM eviction

Layouts are pre-packed on host so every DMA is contiguous-row strided:
    tokT [P, D/P, C]   (d = kc*P + p)
    W1   [P, D/P, F]   (d = kc*P + p)
    W2   [P, F/P, D]   (f = kc*P + p)
    b1t  [P, F/P]      (f = fb*P + p)
    wv   [P, C/CB, CB/P]  (c = cb*CB + m*P + p)
"""
import numpy as np

import concourse.bacc as bacc
import concourse.tile as tile
from concourse import mybir
from concourse.bass_utils import run_bass_kernel_spmd

P = 128
T, D, F, E, C = 8192, 2048, 8192, 8, 2048
CB = 512          # capacity block held resident as hT [F, CB]
NOUT = 512        # output free-dim tile (one PSUM bank of fp32)

f32 = mybir.dt.float32
f32r = mybir.dt.float32r
AF = mybir.ActivationFunctionType


def split_blocks(c_pad):
    """Split c_pad (multiple of 128) into blocks of <=512, each >=256
    (f32r full-rate needs moving free dim >=256)."""
    blocks = []
    rem = c_pad
    while rem > 512:
        blocks.append(512)
        rem -= 512
    if rem == 128 and blocks:
        blocks[-1] = 384
        rem = 256
    blocks.append(rem)
    assert sum(blocks) == c_pad and all(b % 128 == 0 for b in blocks)
    return blocks


def build_nc(d=None, f=None, c_pad=None, nout=None):
    d = D if d is None else d
    f = F if f is None else f
    c_pad = C if c_pad is None else c_pad
    nout = NOUT if nout is None else nout
    KD = d // P       # mm1 contraction chunks
    KF = f // P       # mm2 contraction chunks
    FB = f // P       # mm1 output partition groups
    NB = d // nout    # output col tiles
    blocks = split_blocks(c_pad)
    MT = c_pad // P   # total output row subtiles

    nc = bacc.Bacc()
    tokT = nc.declare_dram_parameter("tokT", [P, KD, c_pad], f32r,
                                     isOutput=False)
    w1 = nc.declare_dram_parameter("w1", [FB, P, KD, P], f32r, isOutput=False)
    w2 = nc.declare_dram_parameter("w2", [NB, KF, P, nout], f32r,
                                    isOutput=False)
    b1t = nc.declare_dram_parameter("b1t", [P, FB], f32, isOutput=False)
    wv = nc.declare_dram_parameter("wv", [P, MT], f32, isOutput=False)
    out = nc.declare_dram_parameter("out", [c_pad, d], f32, isOutput=True)

    with tile.TileContext(nc) as tc:
        with tc.tile_pool(name="const", bufs=1) as const, \
             tc.tile_pool(name="tokp", bufs=1) as tokp, \
             tc.tile_pool(name="hp", bufs=1) as hp, \
             tc.tile_pool(name="w1p", bufs=2) as w1p, \
             tc.tile_pool(name="w2p", bufs=6) as w2p, \
             tc.tile_pool(name="ostp", bufs=2) as ostp, \
             tc.tile_pool(name="php", bufs=3, space="PSUM") as php, \
             tc.tile_pool(name="pop", bufs=1, space="PSUM") as pop:
            b1s = const.tile([P, FB], f32)
            nc.sync.dma_start(b1s[:], b1t[:])
            wvs = const.tile([P, MT], f32)
            nc.sync.dma_start(wvs[:], wv[:])

            c_off = 0
            m_off = 0
            for cb in blocks:
                MB = cb // P
                tok_c = tokp.tile([P, KD, 512], f32r, tag="tok")
                for kq in range(KD):
                    nc.sync.dma_start(tok_c[:, kq, :cb],
                                      tokT[:, kq, c_off:c_off + cb])
                hT = hp.tile([P, KF, 512], f32r, tag="hT")

                # mm1: hT[f, :] = gelu(W1^T @ tokT + b1)
                for fb in range(FB):
                    w1t = w1p.tile([P, KD, P], f32r, tag="w1t")
                    for kq in range(0, KD, 4):
                        nc.sync.dma_start(w1t[:, kq:kq + 4, :],
                                          w1[fb, :, kq:kq + 4, :])
                    ph = php.tile([P, 512], f32, tag="ph")
                    for kc in range(KD):
                        nc.tensor.matmul(ph[:, :cb], w1t[:, kc, :],
                                         tok_c[:, kc, :cb],
                                         start=(kc == 0), stop=(kc == KD - 1))
                    # fast DVE drain of PSUM, then gelu in place on ACT off
                    # the PE critical path
                    nc.vector.tensor_copy(hT[:, fb, :cb], ph[:, :cb])
                    nc.scalar.activation(hT[:, fb, :cb], hT[:, fb, :cb],
                                         AF.Gelu_apprx_tanh,
                                         bias=b1s[:, fb:fb + 1])

                # mm2: out[c, :] = (hT^T @ W2) * w', w' = dup-count * weight
                for nb in range(NB):
                    pos = [pop.tile([P, nout], f32, tag=f"po{m}", name=f"po{m}")
                           for m in range(MB)]
                    for kc in range(KF):
                        w2t = w2p.tile([P, nout], f32r, tag="w2t")
                        nc.sync.dma_start(w2t[:], w2[nb, kc])
                        for m in range(MB):
                            nc.tensor.matmul(pos[m][:],
                                             hT[:, kc, m * P:(m + 1) * P],
                                             w2t[:],
                                             start=(kc == 0),
                                             stop=(kc == KF - 1))
                    for m in range(MB):
                        ost = ostp.tile([P, nout], f32, tag="ost")
                        mg = m_off + m
                        nc.vector.tensor_tensor(
                            ost[:], pos[m][:],
                            wvs[:, mg:mg + 1].to_broadcast((P, nout)),
                            mybir.AluOpType.mult)
                        r0 = c_off + m * P
                        nc.sync.dma_start(
                            out[r0:r0 + P, nb * nout:(nb + 1) * nout], ost[:])
                c_off += cb
                m_off += MB
    nc.compile()
    return nc


def pack_core(inputs, inputs_weight, top_idx, W1, b1, e, c_pad,
              d=None, f=None):
    """Host-side dispatch for expert e: dedup duplicate routed tokens
    (identical rows), fold duplicate counts into the combine weight,
    gather + relayout, pad to c_pad rows (pad weight = 0)."""
    d = D if d is None else d
    f = F if f is None else f
    KD = d // P
    FB = f // P
    idx = np.asarray(top_idx[:, e])
    u, counts = np.unique(idx, return_counts=True)
    n_u = len(u)
    assert n_u <= c_pad
    w_fold = np.zeros(c_pad, dtype=np.float32)
    w_fold[:n_u] = inputs_weight[u, e].astype(np.float32) * counts
    u_pad = np.zeros(c_pad, dtype=idx.dtype)
    u_pad[:n_u] = u
    tok = np.zeros((c_pad, d), dtype=np.float32)
    tok[:n_u] = inputs[u]
    # tokT[p, kc, c] = tok[c, kc*P + p]
    tokT = tok.T.reshape(KD, P, c_pad).transpose(1, 0, 2)
    # w1m[fb, p, kc, j] = W1[kc*P + p, fb*P + j]
    w1m = W1[e].reshape(KD, P, FB, P).transpose(2, 1, 0, 3)
    b1m = np.ascontiguousarray(b1[e]).reshape(FB, P).T
    # wvm[p, mg] for c = mg*P + p
    wvm = w_fold.reshape(c_pad // P, P).T
    return u_pad, n_u, w_fold, tokT, w1m, b1m, wvm


_NC_CACHE = {}


def get_nc(c_pad):
    key = (D, F, c_pad, NOUT)
    if key not in _NC_CACHE:
        _NC_CACHE[key] = build_nc(c_pad=c_pad)
    return _NC_CACHE[key]


def make_in_maps(inputs, inputs_weight, top_idx, W1, b1, W2, b2):
    KF = F // P
    NB = D // NOUT
    # uniform SPMD program: pad every expert to the max unique count
    n_us = [len(np.unique(np.asarray(top_idx[:, e]))) for e in range(E)]
    c_pad = min(C, -(-max(max(n_us), 256) // P) * P)
    in_maps = []
    idxs = []
    wvs = []
    for e in range(E):
        u_pad, n_u, w_fold, tokT, w1m, b1m, wvm = pack_core(
            inputs, inputs_weight, top_idx, W1, b1, e, c_pad)
        # w2m[nb, kc, p, j] = W2[kc*P + p, nb*NOUT + j]
        w2m = W2[e].reshape(KF, P, NB, NOUT).transpose(2, 0, 1, 3)
        in_maps.append({
            "tokT": np.ascontiguousarray(tokT, dtype=np.float32),
            "w1": np.ascontiguousarray(w1m, dtype=np.float32),
            "w2": np.ascontiguousarray(w2m, dtype=np.float32),
            "b1t": np.ascontiguousarray(b1m, dtype=np.float32),
            "wv": np.ascontiguousarray(wvm, dtype=np.float32),
        })
        idxs.append(u_pad)
        wvs.append(w_fold)
    return c_pad, in_maps, idxs, wvs


def combine(outs, idxs, wvs, b2):
    """Host-side combine: weighted scatter-add back to token positions.
    Device rows already carry w' = dup_count * weight; pad rows have w'=0."""
    vals = []
    for e in range(E):
        v = outs[e]
        if np.any(b2[e]):
            v = v + wvs[e][:, None] * b2[e][None, :].astype(np.float32)
        vals.append(v)
    vals = np.concatenate(vals, axis=0)          # [E*c_pad, D]
    idx_all = np.concatenate(idxs, axis=0)       # [E*c_pad]

    order = np.argsort(idx_all, kind="stable")
    si = idx_all[order]
    sv = vals[order]
    starts = np.flatnonzero(np.r_[True, si[1:] != si[:-1]])
    sums = np.add.reduceat(sv, starts, axis=0)
    res = np.zeros((T, D), dtype=np.float32)
    res[si[starts]] = sums
    return res


def kernel(inputs, inputs_weight, top_idx, W1, b1, W2, b2):
    inputs = np.asarray(inputs, dtype=np.float32)
    inputs_weight = np.asarray(inputs_weight, dtype=np.float32)
    top_idx = np.asarray(top_idx)
    W1 = np.asarray(W1, dtype=np.float32)
    b1 = np.asarray(b1, dtype=np.float32)
    W2 = np.asarray(W2, dtype=np.float32)
    b2 = np.asarray(b2, dtype=np.float32)

    c_pad, in_maps, idxs, wvs = make_in_maps(
        inputs, inputs_weight, top_idx, W1, b1, W2, b2)
    nc = get_nc(c_pad)
    try:
        r = run_bass_kernel_spmd(nc, in_maps, list(range(E)))
    except Exception:
        # transient NRT/device hiccups happen; one retry is usually enough
        import time as _time
        _time.sleep(5)
        r = run_bass_kernel_spmd(nc, in_maps, list(range(E)))
    outs = [r.results[e]["out"] for e in range(E)]
    return combine(outs, idxs, wvs, b2)
